# revision 1
# baseline (speedup 1.0000x reference)
"""Multi-head causal attention (B=2, S=2048, D=2048, H=16, HD=128) on 8 TRN2
NeuronCores.

Sharding: data-parallel over batch (2 groups of 4 cores) x tensor-parallel
over heads (4 heads per core).  Each core computes q/k/v projections for its
512 columns (4 heads), causal attention for those heads, and a partial
(contraction-sharded) wo product.  The 4 partial outputs per batch are summed
on the host (the "all-reduce after wo" of the sharding hint).

Everything on-chip is computed in transposed orientation:
  xT [d, s] (host pre-transposed), qT/kT [j, s], scores^T [t, s], out^T [j2, s]
so every matmul contraction lands on the partition axis with zero on-chip
transposes.  Matmuls run as float32r (full-rate fp32 path).  Softmax uses exp
without max-subtraction (scores are O(5), exact in fp32) with denominators
from a ones-vector matmul; causal masking multiplies a precomputed staircase
mask post-exp (exact zeros, matching the reference's exp(-1e9) == 0
underflow).  Fully-masked key blocks are skipped.

The emission is software-pipelined one chunk deep: projections of chunk c are
interleaved with attention of chunk c-1 so the weight streaming (the dominant
DMA) spreads over the whole chunk instead of saturating HBM during a dense
projection burst.
"""

import numpy as np

import concourse.bass as bass
import concourse.tile as tile
from concourse import bacc, mybir
from concourse.bass_utils import run_bass_kernel_spmd

B, S, D = 2, 2048, 2048
H, HD = 16, 128
P = 128
JL = 512          # local q/k/v columns per core (4 heads)
NH = 4            # heads per core
CHUNK = 512       # s-chunk
NCH = S // CHUNK  # 4
DT = D // P       # 16 d-tiles
NT = S // P       # 16 t-tiles
SCALE = 1.0 / float(np.sqrt(HD))

F32 = mybir.dt.float32
F32R = mybir.dt.float32r


def build_kernel():
    nc = bacc.Bacc("TRN2", target_bir_lowering=False, debug=False, num_devices=8)
    xT = nc.dram_tensor("xT", [D, S], F32R, kind="ExternalInput").ap()
    wqT = nc.dram_tensor("wqT", [D, JL], F32R, kind="ExternalInput").ap()
    wkT = nc.dram_tensor("wkT", [D, JL], F32R, kind="ExternalInput").ap()
    wvT = nc.dram_tensor("wvT", [D, JL], F32R, kind="ExternalInput").ap()
    woT = nc.dram_tensor("woT", [JL, D], F32R, kind="ExternalInput").ap()
    outT = nc.dram_tensor("outT", [D, S], F32, kind="ExternalOutput").ap()

    with tile.TileContext(nc) as tc:
        with (
            tc.tile_pool(name="persist", bufs=1) as persist,
            tc.tile_pool(name="xt", bufs=1) as xt_pool,
            tc.tile_pool(name="wst", bufs=6) as wst_pool,
            tc.tile_pool(name="qt", bufs=2) as qt_pool,
            tc.tile_pool(name="exp", bufs=6) as exp_pool,
            tc.tile_pool(name="ot", bufs=4) as ot_pool,
            tc.tile_pool(name="small", bufs=2) as small_pool,
            tc.tile_pool(name="osb", bufs=3) as osb_pool,
            tc.tile_pool(name="ps_main", bufs=1, space="PSUM") as ps_main,
            tc.tile_pool(name="ps_s", bufs=3, space="PSUM") as ps_s,
            tc.tile_pool(name="ps_rs", bufs=1, space="PSUM") as ps_rs,
        ):
            master = persist.tile([P, 896], F32, name="master")
            nc.gpsimd.memset(master[:], 1.0)
            # master[p, u] = 1.0 iff u - p - 384 >= 0 else 0.0
            nc.gpsimd.affine_select(
                out=master[:], in_=master[:], pattern=[[1, 896]],
                compare_op=mybir.AluOpType.is_ge, fill=0.0,
                base=-384, channel_multiplier=-1,
            )
            ones_f = persist.tile([P, 1], F32, name="ones_f")
            nc.vector.memset(ones_f[:], 1.0)
            ones = persist.tile([P, 1], F32R, name="ones")
            nc.vector.tensor_copy(ones[:], ones_f[:])

            kT_t = [persist.tile([P, S], F32R, name=f"kT{h}") for h in range(NH)]
            v_t = [persist.tile([P, JL], F32R, name=f"v{t}") for t in range(NT)]
            woT_t = [persist.tile([P, D], F32R, name=f"woT{h}") for h in range(NH)]
            for h in range(NH):
                nc.scalar.dma_start(out=woT_t[h][:], in_=woT[h * P:(h + 1) * P, :])

            # per-chunk state threaded through the pipeline
            xt_cur = [None]   # xt tiles of the chunk being projected
            qt_of = {}        # chunk -> qt tiles
            ots_of = {}       # chunk -> normalized per-head attention outputs

            def emit_kproj(c):
                ssl = slice(c * CHUNK, (c + 1) * CHUNK)
                ps_k = [ps_main.tile([P, CHUNK], F32, name=f"psk{j}", tag=f"pm{j}")
                        for j in range(4)]
                xt = []
                for d in range(DT):
                    t_ = xt_pool.tile([P, CHUNK], F32R, name=f"xt{d}", tag=f"xt{d}")
                    nc.sync.dma_start(out=t_[:], in_=xT[d * P:(d + 1) * P, ssl])
                    xt.append(t_)
                    wk_d = wst_pool.tile([P, JL], F32R, name=f"wk{d}", tag="wst")
                    nc.sync.dma_start(out=wk_d[:], in_=wkT[d * P:(d + 1) * P, :])
                    for j in range(4):
                        nc.tensor.matmul(
                            ps_k[j][:], wk_d[:, j * P:(j + 1) * P], xt[d][:],
                            start=(d == 0), stop=(d == DT - 1),
                            skip_group_check=True,
                        )
                for j in range(4):
                    nc.vector.tensor_copy(kT_t[j][:, ssl], ps_k[j][:])
                xt_cur[0] = xt

            def emit_qproj(c):
                xt = xt_cur[0]
                ps_q = [ps_main.tile([P, CHUNK], F32, name=f"psq{j}", tag=f"pm{j}")
                        for j in range(4)]
                for d in range(DT):
                    wq_d = wst_pool.tile([P, JL], F32R, name=f"wq{d}", tag="wst")
                    nc.sync.dma_start(out=wq_d[:], in_=wqT[d * P:(d + 1) * P, :])
                    for j in range(4):
                        nc.tensor.matmul(
                            ps_q[j][:], wq_d[:, j * P:(j + 1) * P], xt[d][:],
                            start=(d == 0), stop=(d == DT - 1),
                            skip_group_check=True,
                        )
                qt = []
                for j in range(4):
                    t_ = qt_pool.tile([P, CHUNK], F32R, name=f"qt{j}", tag=f"qt{j}")
                    nc.vector.tensor_copy(t_[:], ps_q[j][:])
                    qt.append(t_)
                qt_of[c] = qt

            def emit_vproj(c):
                xt = xt_cur[0]
                ps_v = [ps_main.tile([P, CHUNK], F32, name=f"psv{i}", tag=f"pm{i}")
                        for i in range(4)]
                for d in range(DT):
                    wv_d = wst_pool.tile([P, JL], F32R, name=f"wv{d}", tag="wst")
                    nc.sync.dma_start(out=wv_d[:], in_=wvT[d * P:(d + 1) * P, :])
                    for i in range(4):
                        nc.tensor.matmul(
                            ps_v[i][:], xt[d][:, i * P:(i + 1) * P], wv_d[:],
                            start=(d == 0), stop=(d == DT - 1),
                            skip_group_check=True,
                        )
                for i in range(4):
                    nc.vector.tensor_copy(v_t[4 * c + i][:], ps_v[i][:])

            def emit_attn_head(c, h):
                s0 = c * CHUNK
                qt = qt_of[c]
                T = 4 * c + 4
                rs_acc = ps_rs.tile([1, CHUNK], F32, name="rsacc", tag="rs")
                o_acc = ps_s.tile([P, CHUNK], F32, name="oacc", tag="ss")
                exps = [None] * T

                def emit_b(t):
                    nc.tensor.matmul(
                        rs_acc[:], ones[:], exps[t][:],
                        start=(t == 0), stop=(t == T - 1),
                        skip_group_check=True,
                    )
                    nc.tensor.matmul(
                        o_acc[:], v_t[t][:, h * P:(h + 1) * P], exps[t][:],
                        start=(t == 0), stop=(t == T - 1),
                        skip_group_check=True,
                    )

                ngroups = T // 4
                for g in range(ngroups):
                    for i in range(4):
                        t = 4 * g + i
                        ps = ps_s.tile([P, CHUNK], F32, name="pss", tag="ss")
                        nc.tensor.matmul(
                            ps[:], kT_t[h][:, t * P:(t + 1) * P], qt[h][:],
                            start=True, stop=True, skip_group_check=True,
                        )
                        e = exp_pool.tile([P, CHUNK], F32R, name="exp", tag="exp")
                        nc.scalar.activation(
                            e[:], ps[:], mybir.ActivationFunctionType.Exp,
                            scale=SCALE,
                        )
                        if t >= 4 * c:
                            off = 384 + s0 - t * P
                            nc.vector.tensor_mul(
                                e[:], e[:], master[:, off:off + CHUNK])
                        exps[t] = e
                    if g >= 1:
                        for i in range(4):
                            emit_b(4 * (g - 1) + i)
                for i in range(4):
                    emit_b(4 * (ngroups - 1) + i)

                # normalize: reciprocal (PSUM->SBUF), broadcast, multiply
                rs_sb = small_pool.tile([1, CHUNK], F32, name="rssb", tag="rssb")
                nc.vector.reciprocal_approx_fast(out=rs_sb[:], in_=rs_acc[:])
                rb = small_pool.tile([P, CHUNK], F32, name="rb", tag="rb")
                nc.gpsimd.partition_broadcast(rb[:], rs_sb[:])
                ot = ot_pool.tile([P, CHUNK], F32R, name="ot", tag="ot")
                nc.vector.tensor_mul(ot[:], o_acc[:], rb[:])
                ots_of.setdefault(c, []).append(ot)

            def emit_wo(c):
                ssl = slice(c * CHUNK, (c + 1) * CHUNK)
                ots = ots_of.pop(c)
                for j2 in range(DT):
                    pw = ps_s.tile([P, CHUNK], F32, name="pw", tag="ss")
                    for h in range(NH):
                        nc.tensor.matmul(
                            pw[:], woT_t[h][:, j2 * P:(j2 + 1) * P], ots[h][:],
                            start=(h == 0), stop=(h == NH - 1),
                            skip_group_check=True,
                        )
                    ob = osb_pool.tile([P, CHUNK], F32, name="ob", tag="ob")
                    nc.vector.tensor_copy(ob[:], pw[:])
                    nc.scalar.dma_start(out=outT[j2 * P:(j2 + 1) * P, ssl],
                                        in_=ob[:])

            # ---- software pipeline: proj(c) interleaved with attn(c-1) ----
            emit_kproj(0)
            emit_qproj(0)
            emit_vproj(0)
            for c in range(1, NCH):
                emit_kproj(c)
                emit_attn_head(c - 1, 0)
                emit_attn_head(c - 1, 1)
                emit_qproj(c)
                emit_attn_head(c - 1, 2)
                emit_attn_head(c - 1, 3)
                emit_vproj(c)
                emit_wo(c - 1)
            for h in range(NH):
                emit_attn_head(NCH - 1, h)
            emit_wo(NCH - 1)

    nc.compile()
    return nc


_NC_CACHE = None


def _get_nc():
    global _NC_CACHE
    if _NC_CACHE is None:
        _NC_CACHE = build_kernel()
    return _NC_CACHE


def make_in_maps(x, wq, wk, wv, wo):
    in_maps = []
    for core in range(8):
        b, g = core // 4, core % 4
        j0 = g * JL
        in_maps.append({
            "xT": np.ascontiguousarray(x[b].T).astype(np.float32, copy=False),
            "wqT": np.ascontiguousarray(wq[j0:j0 + JL, :].T),
            "wkT": np.ascontiguousarray(wk[j0:j0 + JL, :].T),
            "wvT": np.ascontiguousarray(wv[j0:j0 + JL, :].T),
            "woT": np.ascontiguousarray(wo[:, j0:j0 + JL].T),
        })
    return in_maps


def kernel(x, freqs_complex=None, mask=None, wq=None, wk=None, wv=None, wo=None,
           **_unused):
    x = np.asarray(x, dtype=np.float32)
    wq = np.asarray(wq, dtype=np.float32)
    wk = np.asarray(wk, dtype=np.float32)
    wv = np.asarray(wv, dtype=np.float32)
    wo = np.asarray(wo, dtype=np.float32)

    nc = _get_nc()
    in_maps = make_in_maps(x, wq, wk, wv, wo)
    res = run_bass_kernel_spmd(nc, in_maps, list(range(8)))

    out = np.zeros((B, S, D), dtype=np.float32)
    for core in range(8):
        out[core // 4] += res.results[core]["outT"].T
    return out



# revision 2
# speedup vs baseline: 1.0711x; 1.0711x over previous
"""Multi-head causal attention (B=2, S=2048, D=2048, H=16, HD=128) on 8 TRN2
NeuronCores.

Sharding: data-parallel over batch (2 groups of 4 cores) x tensor-parallel
over heads (4 heads per core).  Each core computes q/k/v projections for its
512 columns (4 heads), causal attention for those heads, and a partial
(contraction-sharded) wo product.  The 4 partial outputs per batch are summed
on the host (the "all-reduce after wo" of the sharding hint).

Everything on-chip is computed in transposed orientation:
  xT [d, s] (host pre-transposed), qT/kT [j, s], scores^T [t, s], out^T [j2, s]
so every matmul contraction lands on the partition axis with zero on-chip
transposes.  All matmul operands are bf16 (fp32 PSUM accumulation): bf16
halves LDWEIGHTS time (which otherwise caps the tensor engine below the
512-cycle streaming rate), halves HBM traffic, and lets all four weight
matrices live in SBUF for the whole kernel (loaded once, not per chunk).

Softmax uses exp without max-subtraction (scores are O(5), exact in fp32)
with denominators from a ones-vector matmul; causal masking multiplies a
precomputed staircase mask post-exp (exact zeros, matching the reference's
exp(-1e9) == 0 underflow).  Causality is exploited at 128-key-tile
granularity: diagonal key tiles only stream the valid q columns
(partial-N matmuls), and only the 128-wide partial block gets the mask.

Emission is interleaved at fine grain: attention of chunk c-1 (whose score ->
exp -> PV chain is latency-bound on the scalar engine) is woven between the
q/v projection matmuls of chunk c, so the in-order tensor queue always has
dependency-free projection work while exps drain.  DMA issue runs on the
sync engine (loads) and gpsimd (stores), keeping the scalar engine free for
exp.
"""

import numpy as np
import ml_dtypes

import concourse.bass as bass
import concourse.tile as tile
from concourse import bacc, mybir
from concourse.bass_utils import run_bass_kernel_spmd

B, S, D = 2, 2048, 2048
H, HD = 16, 128
P = 128
JL = 512          # local q/k/v columns per core (4 heads)
NH = 4            # heads per core
CHUNK = 512       # s-chunk
NCH = S // CHUNK  # 4
DT = D // P       # 16 d-tiles
NT = S // P       # 16 t-tiles
SCALE = 1.0 / float(np.sqrt(HD))

F32 = mybir.dt.float32
BF16 = mybir.dt.bfloat16


def build_kernel():
    nc = bacc.Bacc("TRN2", target_bir_lowering=False, debug=False, num_devices=8)
    xT = nc.dram_tensor("xT", [D, S], BF16, kind="ExternalInput").ap()
    wqT = nc.dram_tensor("wqT", [D, JL], BF16, kind="ExternalInput").ap()
    wkT = nc.dram_tensor("wkT", [D, JL], BF16, kind="ExternalInput").ap()
    wvT = nc.dram_tensor("wvT", [D, JL], BF16, kind="ExternalInput").ap()
    woT = nc.dram_tensor("woT", [JL, D], BF16, kind="ExternalInput").ap()
    outT = nc.dram_tensor("outT", [D, S], F32, kind="ExternalOutput").ap()

    with tile.TileContext(nc) as tc:
        with (
            tc.tile_pool(name="persist", bufs=1) as persist,
            tc.tile_pool(name="xt", bufs=2) as xt_pool,
            tc.tile_pool(name="qt", bufs=2) as qt_pool,
            tc.tile_pool(name="exp", bufs=8) as exp_pool,
            tc.tile_pool(name="ot", bufs=5) as ot_pool,
            tc.tile_pool(name="small", bufs=2) as small_pool,
            tc.tile_pool(name="osb", bufs=3) as osb_pool,
            tc.tile_pool(name="ps_main", bufs=1, space="PSUM") as ps_main,
            tc.tile_pool(name="ps_s", bufs=3, space="PSUM") as ps_s,
            tc.tile_pool(name="ps_rs", bufs=1, space="PSUM") as ps_rs,
        ):
            # staircase mask: master[p, u] = 1.0 iff u - p - 384 >= 0 else 0.0
            master_f = persist.tile([P, 896], F32, name="master_f")
            nc.gpsimd.memset(master_f[:], 1.0)
            nc.gpsimd.affine_select(
                out=master_f[:], in_=master_f[:], pattern=[[1, 896]],
                compare_op=mybir.AluOpType.is_ge, fill=0.0,
                base=-384, channel_multiplier=-1,
            )
            master = persist.tile([P, 896], BF16, name="master")
            nc.vector.tensor_copy(master[:], master_f[:])
            ones_f = persist.tile([P, 1], F32, name="ones_f")
            nc.vector.memset(ones_f[:], 1.0)
            ones = persist.tile([P, 1], BF16, name="ones")
            nc.vector.tensor_copy(ones[:], ones_f[:])

            # persistent weights (bf16, loaded once)
            wq_t = [persist.tile([P, JL], BF16, name=f"wq{d}") for d in range(DT)]
            wk_t = [persist.tile([P, JL], BF16, name=f"wk{d}") for d in range(DT)]
            wv_t = [persist.tile([P, JL], BF16, name=f"wv{d}") for d in range(DT)]
            wo_t = [persist.tile([P, D], BF16, name=f"wo{h}") for h in range(NH)]
            kT_t = [persist.tile([P, S], BF16, name=f"kT{h}") for h in range(NH)]
            v_t = [persist.tile([P, JL], BF16, name=f"v{t}") for t in range(NT)]

            # wk is needed first; wq/wv/wo stream in during the first chunk's
            # projections (see gen_kproj/gen_qv with c == 0).
            for d in range(DT):
                nc.sync.dma_start(out=wk_t[d][:], in_=wkT[d * P:(d + 1) * P, :])

            xt_cur = [None]   # xt tiles of the chunk being projected
            qt_of = {}        # chunk -> qt tiles
            ots_of = {}       # chunk -> normalized per-head attention outputs

            def gen_kproj(c):
                ssl = slice(c * CHUNK, (c + 1) * CHUNK)
                ps_k = [ps_main.tile([P, CHUNK], F32, name=f"psk{j}", tag=f"pm{j}")
                        for j in range(4)]
                xt = []
                for d in range(DT):
                    t_ = xt_pool.tile([P, CHUNK], BF16, name=f"xt{d}", tag=f"xt{d}")
                    nc.sync.dma_start(out=t_[:], in_=xT[d * P:(d + 1) * P, ssl])
                    if c == 0:
                        nc.sync.dma_start(out=wq_t[d][:],
                                          in_=wqT[d * P:(d + 1) * P, :])
                    xt.append(t_)
                    for j in range(4):
                        nc.tensor.matmul(
                            ps_k[j][:], wk_t[d][:, j * P:(j + 1) * P], t_[:],
                            start=(d == 0), stop=(d == DT - 1),
                            skip_group_check=True,
                        )
                    yield
                for j in range(4):
                    nc.vector.tensor_copy(kT_t[j][:, ssl], ps_k[j][:])
                xt_cur[0] = xt
                yield

            def gen_qv(c):
                # qproj then vproj of chunk c; 34 yields
                xt = xt_cur[0]
                ps_q = [ps_main.tile([P, CHUNK], F32, name=f"psq{j}", tag=f"pm{j}")
                        for j in range(4)]
                for d in range(DT):
                    for j in range(4):
                        nc.tensor.matmul(
                            ps_q[j][:], wq_t[d][:, j * P:(j + 1) * P], xt[d][:],
                            start=(d == 0), stop=(d == DT - 1),
                            skip_group_check=True,
                        )
                    if c == 0:
                        nc.sync.dma_start(out=wv_t[d][:],
                                          in_=wvT[d * P:(d + 1) * P, :])
                    yield
                qt = []
                for j in range(4):
                    t_ = qt_pool.tile([P, CHUNK], BF16, name=f"qt{j}", tag=f"qt{j}")
                    nc.vector.tensor_copy(t_[:], ps_q[j][:])
                    qt.append(t_)
                qt_of[c] = qt
                yield
                ps_v = [ps_main.tile([P, CHUNK], F32, name=f"psv{i}", tag=f"pm{i}")
                        for i in range(4)]
                for d in range(DT):
                    for i in range(4):
                        nc.tensor.matmul(
                            ps_v[i][:], xt[d][:, i * P:(i + 1) * P], wv_t[d][:],
                            start=(d == 0), stop=(d == DT - 1),
                            skip_group_check=True,
                        )
                    if c == 0 and d < NH:
                        nc.gpsimd.dma_start(out=wo_t[d][:],
                                            in_=woT[d * P:(d + 1) * P, :])
                    yield
                for i in range(4):
                    nc.vector.tensor_copy(v_t[4 * c + i][:], ps_v[i][:])
                yield

            def gen_attn_head(c, h):
                # 4G+1 yields, G = c+1
                qt = qt_of[c]
                T = 4 * c + 4
                G = T // 4
                rs_acc = ps_rs.tile([1, CHUNK], F32, name="rsacc", tag="rs")
                o_acc = ps_s.tile([P, CHUNK], F32, name="oacc", tag="ss")
                exps = [None] * T

                def emit_scores(t):
                    u = t - 4 * c
                    q0 = u * P if u > 0 else 0
                    ps = ps_s.tile([P, CHUNK], F32, name="pss", tag="ss")
                    nc.tensor.matmul(
                        ps[:, q0:], kT_t[h][:, t * P:(t + 1) * P], qt[h][:, q0:],
                        start=True, stop=True, skip_group_check=True,
                    )
                    e = exp_pool.tile([P, CHUNK], BF16, name="exp", tag="exp")
                    nc.scalar.activation(
                        e[:, q0:], ps[:, q0:], mybir.ActivationFunctionType.Exp,
                        scale=SCALE,
                    )
                    if u >= 0:
                        nc.vector.tensor_mul(
                            e[:, q0:q0 + P], e[:, q0:q0 + P], master[:, 384:384 + P])
                    exps[t] = (e, q0)

                def emit_b(t):
                    e, q0 = exps[t]
                    nc.tensor.matmul(
                        rs_acc[:, q0:], ones[:], e[:, q0:],
                        start=(t == 0), stop=(t == T - 1),
                        skip_group_check=True,
                    )
                    nc.tensor.matmul(
                        o_acc[:, q0:], v_t[t][:, h * P:(h + 1) * P], e[:, q0:],
                        start=(t == 0), stop=(t == T - 1),
                        skip_group_check=True,
                    )

                for g in range(G):
                    emit_scores(4 * g); emit_scores(4 * g + 1)
                    yield
                    emit_scores(4 * g + 2); emit_scores(4 * g + 3)
                    yield
                    if g >= 1:
                        emit_b(4 * (g - 1)); emit_b(4 * (g - 1) + 1)
                        yield
                        emit_b(4 * (g - 1) + 2); emit_b(4 * (g - 1) + 3)
                        yield
                emit_b(4 * (G - 1)); emit_b(4 * (G - 1) + 1)
                yield
                emit_b(4 * (G - 1) + 2); emit_b(4 * (G - 1) + 3)

                # normalize: reciprocal (PSUM->SBUF), broadcast, multiply
                rs_sb = small_pool.tile([1, CHUNK], F32, name="rssb", tag="rssb")
                nc.vector.reciprocal_approx_fast(out=rs_sb[:], in_=rs_acc[:])
                rb = small_pool.tile([P, CHUNK], F32, name="rb", tag="rb")
                nc.gpsimd.partition_broadcast(rb[:], rs_sb[:])
                ot = ot_pool.tile([P, CHUNK], BF16, name="ot", tag="ot")
                nc.vector.tensor_mul(ot[:], o_acc[:], rb[:])
                ots_of.setdefault(c, []).append(ot)
                yield

            def gen_attn(c):
                for h in range(NH):
                    yield from gen_attn_head(c, h)

            def gen_wo(c):
                ssl = slice(c * CHUNK, (c + 1) * CHUNK)
                ots = ots_of.pop(c)
                for j2 in range(DT):
                    pw = ps_s.tile([P, CHUNK], F32, name="pw", tag="ss")
                    for h in range(NH):
                        nc.tensor.matmul(
                            pw[:], wo_t[h][:, j2 * P:(j2 + 1) * P], ots[h][:],
                            start=(h == 0), stop=(h == NH - 1),
                            skip_group_check=True,
                        )
                    ob = osb_pool.tile([P, CHUNK], F32, name="ob", tag="ob")
                    nc.vector.tensor_copy(ob[:], pw[:])
                    nc.gpsimd.dma_start(out=outT[j2 * P:(j2 + 1) * P, ssl],
                                        in_=ob[:])
                    yield

            def drive(g):
                for _ in g:
                    pass

            SENT = object()

            def interleave(gmain, nmain, gsub, nsub):
                im = isub = 0
                main_done = sub_done = False
                while not (main_done and sub_done):
                    go_main = sub_done or (
                        not main_done and im * nsub <= isub * nmain)
                    if go_main:
                        if next(gmain, SENT) is SENT:
                            main_done = True
                        else:
                            im += 1
                    else:
                        if next(gsub, SENT) is SENT:
                            sub_done = True
                        else:
                            isub += 1

            # ---- schedule ----
            drive(gen_kproj(0))
            drive(gen_qv(0))
            for c in range(1, NCH):
                drive(gen_kproj(c))
                interleave(gen_qv(c), 34, gen_attn(c - 1), 16 * c + 4)
                drive(gen_wo(c - 1))
            drive(gen_attn(NCH - 1))
            drive(gen_wo(NCH - 1))

    nc.compile()
    return nc


_NC_CACHE = None


def _get_nc():
    global _NC_CACHE
    if _NC_CACHE is None:
        _NC_CACHE = build_kernel()
    return _NC_CACHE


def make_in_maps(x, wq, wk, wv, wo):
    bf16 = ml_dtypes.bfloat16
    in_maps = []
    for core in range(8):
        b, g = core // 4, core % 4
        j0 = g * JL
        in_maps.append({
            "xT": x[b].T.astype(bf16),
            "wqT": wq[j0:j0 + JL, :].T.astype(bf16),
            "wkT": wk[j0:j0 + JL, :].T.astype(bf16),
            "wvT": wv[j0:j0 + JL, :].T.astype(bf16),
            "woT": wo[:, j0:j0 + JL].T.astype(bf16),
        })
    return in_maps


def kernel(x, freqs_complex=None, mask=None, wq=None, wk=None, wv=None, wo=None,
           **_unused):
    x = np.asarray(x, dtype=np.float32)
    wq = np.asarray(wq, dtype=np.float32)
    wk = np.asarray(wk, dtype=np.float32)
    wv = np.asarray(wv, dtype=np.float32)
    wo = np.asarray(wo, dtype=np.float32)

    nc = _get_nc()
    in_maps = make_in_maps(x, wq, wk, wv, wo)
    res = run_bass_kernel_spmd(nc, in_maps, list(range(8)))

    out = np.zeros((B, S, D), dtype=np.float32)
    for core in range(8):
        out[core // 4] += res.results[core]["outT"].T
    return out


# revision 13
# speedup vs baseline: 1.1652x; 1.0879x over previous
"""Multi-head causal attention (B=2, S=2048, D=2048, H=16, HD=128) on 8 TRN2
NeuronCores.

Sharding: data-parallel over batch (2 groups of 4 cores) x tensor-parallel
over heads (4 heads per core).  Each core computes q/k/v projections for its
512 columns (4 heads), causal attention for those heads, and a partial
(contraction-sharded) wo product.  The 4 partial outputs per batch are summed
on the host (the "all-reduce after wo" of the sharding hint).

Everything on-chip is computed in transposed orientation:
  xT [d, s] (host pre-transposed), qT/kT [j, s], scores^T [t, s], out^T [j2, s]
so every matmul contraction lands on the partition axis with zero on-chip
transposes.  All matmul operands are bf16 (fp32 PSUM accumulation): bf16
halves LDWEIGHTS time (which otherwise caps the tensor engine below the
512-cycle streaming rate), halves HBM traffic, and lets all four weight
matrices live in SBUF for the whole kernel (loaded once, not per chunk).

Softmax uses exp without max-subtraction (scores are O(5), exact in fp32)
with denominators from a ones-vector matmul; causal masking multiplies a
precomputed staircase mask post-exp (exact zeros, matching the reference's
exp(-1e9) == 0 underflow).  Causality is exploited at 128-key-tile
granularity: diagonal key tiles only stream the valid q columns
(partial-N matmuls), and only the 128-wide partial block gets the mask.

Emission is interleaved at fine grain: attention of chunk c-1 (whose score ->
exp -> PV chain is latency-bound on the scalar engine) is woven between the
q/v projection matmuls of chunk c, so the in-order tensor queue always has
dependency-free projection work while exps drain.  DMA issue runs on the
sync engine (loads) and gpsimd (stores), keeping the scalar engine free for
exp.
"""

import numpy as np
import ml_dtypes

import concourse.bass as bass
import concourse.tile as tile
from concourse import bacc, mybir
from concourse.bass_utils import run_bass_kernel_spmd

B, S, D = 2, 2048, 2048
H, HD = 16, 128
P = 128
JL = 512          # local q/k/v columns per core (4 heads)
NH = 4            # heads per core
CHUNK = 512       # s-chunk
NCH = S // CHUNK  # 4
DT = D // P       # 16 d-tiles
NT = S // P       # 16 t-tiles
SCALE = 1.0 / float(np.sqrt(HD))

F32 = mybir.dt.float32
F32R = mybir.dt.float32r
BF16 = mybir.dt.bfloat16


def build_kernel():
    nc = bacc.Bacc("TRN2", target_bir_lowering=False, debug=False, num_devices=8)
    xT = nc.dram_tensor("xT", [D, S], BF16, kind="ExternalInput").ap()
    wqT = nc.dram_tensor("wqT", [D, JL], BF16, kind="ExternalInput").ap()
    wkT = nc.dram_tensor("wkT", [D, JL], BF16, kind="ExternalInput").ap()
    wvT = nc.dram_tensor("wvT", [D, JL], BF16, kind="ExternalInput").ap()
    woT = nc.dram_tensor("woT", [JL, D], BF16, kind="ExternalInput").ap()
    outT = nc.dram_tensor("outT", [D, S], BF16, kind="ExternalOutput").ap()

    with tile.TileContext(nc) as tc:
        with (
            tc.tile_pool(name="persist", bufs=1) as persist,
            tc.tile_pool(name="xt", bufs=2) as xt_pool,
            tc.tile_pool(name="qt", bufs=2) as qt_pool,
            tc.tile_pool(name="exp", bufs=8) as exp_pool,
            tc.tile_pool(name="ot", bufs=5) as ot_pool,
            tc.tile_pool(name="esum", bufs=2) as esum_pool,
            tc.tile_pool(name="small", bufs=2) as small_pool,
            tc.tile_pool(name="osb", bufs=3) as osb_pool,
            tc.tile_pool(name="ps_main", bufs=1, space="PSUM") as ps_main,
            tc.tile_pool(name="ps_s", bufs=3, space="PSUM") as ps_s,
            tc.tile_pool(name="ps_rs", bufs=1, space="PSUM") as ps_rs,
        ):
            # staircase mask: master[p, u] = 1.0 iff u - p - 384 >= 0 else 0.0
            master_f = persist.tile([P, 896], F32, name="master_f")
            nc.gpsimd.memset(master_f[:], 1.0)
            nc.gpsimd.affine_select(
                out=master_f[:], in_=master_f[:], pattern=[[1, 896]],
                compare_op=mybir.AluOpType.is_ge, fill=0.0,
                base=-384, channel_multiplier=-1,
            )
            master = persist.tile([P, 896], BF16, name="master")
            nc.vector.tensor_copy(master[:], master_f[:])
            ones_f = persist.tile([P, 1], F32, name="ones_f")
            nc.vector.memset(ones_f[:], 1.0)
            ones = persist.tile([P, 1], BF16, name="ones")
            nc.vector.tensor_copy(ones[:], ones_f[:])
            ones_r = persist.tile([P, 1], F32R, name="ones_r")
            nc.vector.tensor_copy(ones_r[:], ones_f[:])

            # persistent weights (bf16, loaded once)
            wq_t = [persist.tile([P, JL], BF16, name=f"wq{d}") for d in range(DT)]
            wk_t = [persist.tile([P, JL], BF16, name=f"wk{d}") for d in range(DT)]
            wv_t = [persist.tile([P, JL], BF16, name=f"wv{d}") for d in range(DT)]
            wo_t = [persist.tile([P, D], BF16, name=f"wo{h}") for h in range(NH)]
            kT_t = [persist.tile([P, S], BF16, name=f"kT{h}") for h in range(NH)]
            v_t = [persist.tile([P, JL], BF16, name=f"v{t}") for t in range(NT)]

            # wk/wq stream in during the first chunk's kproj, wv during its
            # qproj, wo during its vproj (see c == 0 branches below).
            xt_cur = [None]   # xt tiles of the chunk being projected
            qt_of = {}        # chunk -> qt tiles
            ots_of = {}       # chunk -> normalized per-head attention outputs

            def gen_kproj(c):
                ssl = slice(c * CHUNK, (c + 1) * CHUNK)
                ps_k = [ps_main.tile([P, CHUNK], F32, name=f"psk{j}", tag=f"pm{j}")
                        for j in range(4)]
                xt = []
                for d in range(DT):
                    if c == 0:
                        nc.sync.dma_start(out=wk_t[d][:],
                                          in_=wkT[d * P:(d + 1) * P, :])
                    t_ = xt_pool.tile([P, CHUNK], BF16, name=f"xt{d}", tag=f"xt{d}")
                    nc.sync.dma_start(out=t_[:], in_=xT[d * P:(d + 1) * P, ssl])
                    if c == 0:
                        nc.sync.dma_start(out=wq_t[d][:],
                                          in_=wqT[d * P:(d + 1) * P, :])
                    xt.append(t_)
                    for j in range(4):
                        nc.tensor.matmul(
                            ps_k[j][:], wk_t[d][:, j * P:(j + 1) * P], t_[:],
                            start=(d == 0), stop=(d == DT - 1),
                            skip_group_check=True,
                        )
                        if d == DT - 1:
                            # stagger: cast j overlaps the remaining matmuls
                            nc.vector.tensor_copy(kT_t[j][:, ssl], ps_k[j][:])
                    yield
                xt_cur[0] = xt
                yield

            def gen_qv(c):
                # qproj then vproj of chunk c; 34 yields
                xt = xt_cur[0]
                ps_q = [ps_main.tile([P, CHUNK], F32, name=f"psq{j}", tag=f"pm{j}")
                        for j in range(4)]
                qt = []
                for d in range(DT):
                    for j in range(4):
                        nc.tensor.matmul(
                            ps_q[j][:], wq_t[d][:, j * P:(j + 1) * P], xt[d][:],
                            start=(d == 0), stop=(d == DT - 1),
                            skip_group_check=True,
                        )
                        if d == DT - 1:
                            t_ = qt_pool.tile([P, CHUNK], BF16, name=f"qt{j}",
                                              tag=f"qt{j}")
                            nc.vector.tensor_copy(t_[:], ps_q[j][:])
                            qt.append(t_)
                    if c == 0:
                        nc.sync.dma_start(out=wv_t[d][:],
                                          in_=wvT[d * P:(d + 1) * P, :])
                    yield
                qt_of[c] = qt
                ps_v = [ps_main.tile([P, CHUNK], F32, name=f"psv{i}", tag=f"pm{i}")
                        for i in range(4)]
                for d in range(DT):
                    for i in range(4):
                        nc.tensor.matmul(
                            ps_v[i][:], xt[d][:, i * P:(i + 1) * P], wv_t[d][:],
                            start=(d == 0), stop=(d == DT - 1),
                            skip_group_check=True,
                        )
                        if d == DT - 1:
                            nc.vector.tensor_copy(v_t[4 * c + i][:], ps_v[i][:])
                    if c == 0 and d < NH:
                        nc.gpsimd.dma_start(out=wo_t[d][:],
                                            in_=woT[d * P:(d + 1) * P, :])
                    yield
                yield

            def gen_attn_head(c, h):
                # 4G+1 yields, G = c+1
                qt = qt_of[c]
                T = 4 * c + 4
                G = T // 4
                NFULL = 4 * c  # full-width key tiles (before the diagonal)
                rs_acc = ps_rs.tile([1, CHUNK], F32, name="rsacc", tag="rs")
                o_acc = ps_s.tile([P, CHUNK], F32, name="oacc", tag="ss")
                exps = [None] * T
                esum = [None]  # running f32 sum of full-tile exps (vector)

                def emit_scores(t):
                    u = t - 4 * c
                    q0 = u * P if u > 0 else 0
                    ps = ps_s.tile([P, CHUNK], F32, name="pss", tag="ss")
                    nc.tensor.matmul(
                        ps[:, q0:], kT_t[h][:, t * P:(t + 1) * P], qt[h][:, q0:],
                        start=True, stop=True, skip_group_check=True,
                    )
                    e = exp_pool.tile([P, CHUNK], BF16, name="exp", tag="exp")
                    nc.scalar.activation(
                        e[:, q0:], ps[:, q0:], mybir.ActivationFunctionType.Exp,
                        scale=SCALE,
                    )
                    if u >= 0:
                        nc.vector.tensor_mul(
                            e[:, q0:q0 + P], e[:, q0:q0 + P], master[:, 384:384 + P])
                    exps[t] = (e, q0)
                    # fold full-width tiles into the vector-engine exp sum; the
                    # denominator then needs a single ones-matmul instead of
                    # one per key tile.
                    if u < 0 and t >= 1:
                        s_ = esum_pool.tile([P, CHUNK], F32R, name="esum",
                                            tag="esum")
                        if t == 1:
                            nc.vector.tensor_add(s_[:], exps[0][0][:], e[:])
                        else:
                            nc.vector.tensor_add(s_[:], esum[0][:], e[:])
                        esum[0] = s_

                def emit_b(t):
                    e, q0 = exps[t]
                    if t >= NFULL:
                        nc.tensor.matmul(
                            rs_acc[:, q0:], ones[:], e[:, q0:],
                            start=(t == 0), stop=(t == T - 1),
                            skip_group_check=True,
                        )
                    nc.tensor.matmul(
                        o_acc[:, q0:], v_t[t][:, h * P:(h + 1) * P], e[:, q0:],
                        start=(t == 0), stop=(t == T - 1),
                        skip_group_check=True,
                    )

                for g in range(G):
                    emit_scores(4 * g); emit_scores(4 * g + 1)
                    yield
                    emit_scores(4 * g + 2); emit_scores(4 * g + 3)
                    yield
                    if g >= 1:
                        emit_b(4 * (g - 1)); emit_b(4 * (g - 1) + 1)
                        yield
                        emit_b(4 * (g - 1) + 2); emit_b(4 * (g - 1) + 3)
                        yield
                if NFULL > 0:
                    nc.tensor.matmul(
                        rs_acc[:], ones_r[:], esum[0][:],
                        start=True, stop=False, skip_group_check=True,
                    )
                emit_b(4 * (G - 1)); emit_b(4 * (G - 1) + 1)
                yield
                emit_b(4 * (G - 1) + 2); emit_b(4 * (G - 1) + 3)

                # normalize: reciprocal (PSUM->SBUF), broadcast, multiply
                rs_sb = small_pool.tile([1, CHUNK], F32, name="rssb", tag="rssb")
                nc.vector.reciprocal_approx_fast(out=rs_sb[:], in_=rs_acc[:])
                rb = small_pool.tile([P, CHUNK], F32, name="rb", tag="rb")
                nc.gpsimd.partition_broadcast(rb[:], rs_sb[:])
                ot = ot_pool.tile([P, CHUNK], BF16, name="ot", tag="ot")
                nc.vector.tensor_mul(ot[:], o_acc[:], rb[:])
                ots_of.setdefault(c, []).append(ot)
                yield

            def gen_attn(c):
                for h in range(NH):
                    yield from gen_attn_head(c, h)

            def gen_wo(c):
                ssl = slice(c * CHUNK, (c + 1) * CHUNK)
                ots = ots_of.pop(c)
                for j2 in range(DT):
                    pw = ps_s.tile([P, CHUNK], F32, name="pw", tag="ss")
                    for h in range(NH):
                        nc.tensor.matmul(
                            pw[:], wo_t[h][:, j2 * P:(j2 + 1) * P], ots[h][:],
                            start=(h == 0), stop=(h == NH - 1),
                            skip_group_check=True,
                        )
                    ob = osb_pool.tile([P, CHUNK], BF16, name="ob", tag="ob")
                    nc.vector.tensor_copy(ob[:], pw[:])
                    nc.gpsimd.dma_start(out=outT[j2 * P:(j2 + 1) * P, ssl],
                                        in_=ob[:])
                    yield

            def drive(g):
                for _ in g:
                    pass

            SENT = object()

            def interleave(gmain, nmain, gsub, nsub):
                im = isub = 0
                main_done = sub_done = False
                while not (main_done and sub_done):
                    go_main = sub_done or (
                        not main_done and im * nsub <= isub * nmain)
                    if go_main:
                        if next(gmain, SENT) is SENT:
                            main_done = True
                        else:
                            im += 1
                    else:
                        if next(gsub, SENT) is SENT:
                            sub_done = True
                        else:
                            isub += 1

            # ---- schedule ----
            drive(gen_kproj(0))
            drive(gen_qv(0))
            for c in range(1, NCH):
                drive(gen_kproj(c))
                interleave(gen_qv(c), 33, gen_attn(c - 1), 16 * c + 4)
                drive(gen_wo(c - 1))
            drive(gen_attn(NCH - 1))
            drive(gen_wo(NCH - 1))

    nc.compile()
    return nc


_NC_CACHE = None


def _get_nc():
    global _NC_CACHE
    if _NC_CACHE is None:
        _NC_CACHE = build_kernel()
    return _NC_CACHE


def make_in_maps(x, wq, wk, wv, wo):
    bf16 = ml_dtypes.bfloat16
    in_maps = []
    for core in range(8):
        b, g = core // 4, core % 4
        j0 = g * JL
        in_maps.append({
            "xT": x[b].T.astype(bf16),
            "wqT": wq[j0:j0 + JL, :].T.astype(bf16),
            "wkT": wk[j0:j0 + JL, :].T.astype(bf16),
            "wvT": wv[j0:j0 + JL, :].T.astype(bf16),
            "woT": wo[:, j0:j0 + JL].T.astype(bf16),
        })
    return in_maps


def kernel(x, freqs_complex=None, mask=None, wq=None, wk=None, wv=None, wo=None,
           **_unused):
    x = np.asarray(x, dtype=np.float32)
    wq = np.asarray(wq, dtype=np.float32)
    wk = np.asarray(wk, dtype=np.float32)
    wv = np.asarray(wv, dtype=np.float32)
    wo = np.asarray(wo, dtype=np.float32)

    nc = _get_nc()
    in_maps = make_in_maps(x, wq, wk, wv, wo)
    res = run_bass_kernel_spmd(nc, in_maps, list(range(8)))

    out = np.zeros((B, S, D), dtype=np.float32)
    for core in range(8):
        out[core // 4] += res.results[core]["outT"].T.astype(np.float32)
    return out


# revision 14
# speedup vs baseline: 1.4593x; 1.2524x over previous
"""Multi-head causal attention (B=2, S=2048, D=2048, H=16, HD=128) on 8 TRN2
NeuronCores.

Sharding: data-parallel over batch (2 groups of 4 cores) x tensor-parallel
over heads (4 heads per core).  Each core computes q/k/v projections for its
512 columns (4 heads), causal attention for those heads, and a partial
(contraction-sharded) wo product.  The 4 partial outputs per batch are summed
on the host (the "all-reduce after wo" of the sharding hint).

Everything on-chip is computed in transposed orientation:
  xT [d, s] (host pre-transposed), qT/kT [j, s], scores^T [t, s], out^T [j2, s]
so every matmul contraction lands on the partition axis with zero on-chip
transposes.  All matmul operands are bf16 (fp32 PSUM accumulation): bf16
halves LDWEIGHTS time (which otherwise caps the tensor engine below the
512-cycle streaming rate), halves HBM traffic, and lets all four weight
matrices live in SBUF for the whole kernel (loaded once, not per chunk).

All DRAM tensors are host-side pre-tiled to [128, *] partition-major layout
so every DMA moves multi-KB contiguous lines per partition with a handful of
dma_start instructions (dma_start issue costs ~0.7us of engine time each, so
many small DMAs are issue-rate-bound, not bandwidth-bound).

Softmax uses exp without max-subtraction (scores are O(5), exact in fp32)
with causal masking via a precomputed staircase mask post-exp (exact zeros,
matching the reference's exp(-1e9) == 0 underflow).  Causality is exploited
at 128-key-tile granularity: diagonal key tiles only stream the valid q
columns (partial-N matmuls).  Softmax denominators: full-width exp tiles are
summed on the vector engine and reduced with a single ones-matmul per
(chunk, head); only diagonal tiles use individual ones-matmuls.

Emission is interleaved at fine grain: attention of chunk c-1 (whose score ->
exp -> PV chain is latency-bound on the scalar engine) is woven between the
q/v projection matmuls of chunk c, so the in-order tensor queue always has
dependency-free projection work while exps drain.  DMA issue runs on the
sync engine (loads) and gpsimd (stores), keeping the scalar engine free for
exp.
"""

import numpy as np
import ml_dtypes

import concourse.bass as bass
import concourse.tile as tile
from concourse import bacc, mybir
from concourse.bass_utils import run_bass_kernel_spmd

B, S, D = 2, 2048, 2048
H, HD = 16, 128
P = 128
JL = 512          # local q/k/v columns per core (4 heads)
NH = 4            # heads per core
CHUNK = 512       # s-chunk
NCH = S // CHUNK  # 4
DT = D // P       # 16 d-tiles
NT = S // P       # 16 t-tiles
SCALE = 1.0 / float(np.sqrt(HD))
XW = DT * CHUNK   # 8192: one chunk of x / out, tiled
WW = DT * JL      # 8192: one qkv weight, tiled

F32 = mybir.dt.float32
F32R = mybir.dt.float32r
BF16 = mybir.dt.bfloat16


def build_kernel():
    nc = bacc.Bacc("TRN2", target_bir_lowering=False, debug=False, num_devices=8)
    # all pre-tiled [128, *]; see make_in_maps for layouts
    xT = nc.dram_tensor("xT", [P, NCH * XW], BF16, kind="ExternalInput").ap()
    wqT = nc.dram_tensor("wqT", [P, WW], BF16, kind="ExternalInput").ap()
    wkT = nc.dram_tensor("wkT", [P, WW], BF16, kind="ExternalInput").ap()
    wvT = nc.dram_tensor("wvT", [P, WW], BF16, kind="ExternalInput").ap()
    woT = nc.dram_tensor("woT", [P, NH * D], BF16, kind="ExternalInput").ap()
    outT = nc.dram_tensor("outT", [P, NCH * XW], BF16, kind="ExternalOutput").ap()

    with tile.TileContext(nc) as tc:
        with (
            tc.tile_pool(name="persist", bufs=1) as persist,
            tc.tile_pool(name="xt", bufs=2) as xt_pool,
            tc.tile_pool(name="qt", bufs=2) as qt_pool,
            tc.tile_pool(name="exp", bufs=8) as exp_pool,
            tc.tile_pool(name="ot", bufs=5) as ot_pool,
            tc.tile_pool(name="esum", bufs=2) as esum_pool,
            tc.tile_pool(name="small", bufs=2) as small_pool,
            tc.tile_pool(name="osb", bufs=1) as osb_pool,
            tc.tile_pool(name="ps_main", bufs=1, space="PSUM") as ps_main,
            tc.tile_pool(name="ps_s", bufs=3, space="PSUM") as ps_s,
            tc.tile_pool(name="ps_rs", bufs=1, space="PSUM") as ps_rs,
        ):
            # staircase mask: master[p, u] = 1.0 iff u - p - 384 >= 0 else 0.0
            master_f = persist.tile([P, 896], F32, name="master_f")
            nc.gpsimd.memset(master_f[:], 1.0)
            nc.gpsimd.affine_select(
                out=master_f[:], in_=master_f[:], pattern=[[1, 896]],
                compare_op=mybir.AluOpType.is_ge, fill=0.0,
                base=-384, channel_multiplier=-1,
            )
            master = persist.tile([P, 896], BF16, name="master")
            nc.vector.tensor_copy(master[:], master_f[:])
            ones_f = persist.tile([P, 1], F32, name="ones_f")
            nc.vector.memset(ones_f[:], 1.0)
            ones = persist.tile([P, 1], BF16, name="ones")
            nc.vector.tensor_copy(ones[:], ones_f[:])
            ones_r = persist.tile([P, 1], F32R, name="ones_r")
            nc.vector.tensor_copy(ones_r[:], ones_f[:])

            # persistent weights (bf16, loaded once during chunk-0 work)
            wq_all = persist.tile([P, WW], BF16, name="wq")
            wk_all = persist.tile([P, WW], BF16, name="wk")
            wv_all = persist.tile([P, WW], BF16, name="wv")
            wo_all = persist.tile([P, NH * D], BF16, name="wo")
            kT_t = [persist.tile([P, S], BF16, name=f"kT{h}") for h in range(NH)]
            v_t = [persist.tile([P, JL], BF16, name=f"v{t}") for t in range(NT)]

            xt_cur = [None]   # xt tile of the chunk being projected
            qt_of = {}        # chunk -> qt tiles
            ots_of = {}       # chunk -> normalized per-head attention outputs

            QW = 4 * CHUNK    # 2048: one DMA slice = 4 d-tiles

            def gen_kproj(c):
                ssl = slice(c * CHUNK, (c + 1) * CHUNK)
                ps_k = [ps_main.tile([P, CHUNK], F32, name=f"psk{j}", tag=f"pm{j}")
                        for j in range(4)]
                xt = xt_pool.tile([P, XW], BF16, name="xt", tag="xt")
                for d in range(DT):
                    if d % 4 == 0:
                        k = d // 4
                        qsl = slice(k * QW, (k + 1) * QW)
                        if c == 0:
                            nc.sync.dma_start(out=wk_all[:, qsl], in_=wkT[:, qsl])
                        nc.sync.dma_start(out=xt[:, qsl],
                                          in_=xT[:, c * XW + k * QW:
                                                 c * XW + (k + 1) * QW])
                        if c == 0:
                            nc.sync.dma_start(out=wq_all[:, qsl], in_=wqT[:, qsl])
                    for j in range(4):
                        nc.tensor.matmul(
                            ps_k[j][:],
                            wk_all[:, d * JL + j * P:d * JL + (j + 1) * P],
                            xt[:, d * CHUNK:(d + 1) * CHUNK],
                            start=(d == 0), stop=(d == DT - 1),
                            skip_group_check=True,
                        )
                        if d == DT - 1:
                            # stagger: cast j overlaps the remaining matmuls
                            nc.vector.tensor_copy(kT_t[j][:, ssl], ps_k[j][:])
                    yield
                xt_cur[0] = xt
                yield

            def gen_qv(c):
                # qproj then vproj of chunk c; 33 yields
                xt = xt_cur[0]
                ps_q = [ps_main.tile([P, CHUNK], F32, name=f"psq{j}", tag=f"pm{j}")
                        for j in range(4)]
                qt = []
                for d in range(DT):
                    for j in range(4):
                        nc.tensor.matmul(
                            ps_q[j][:],
                            wq_all[:, d * JL + j * P:d * JL + (j + 1) * P],
                            xt[:, d * CHUNK:(d + 1) * CHUNK],
                            start=(d == 0), stop=(d == DT - 1),
                            skip_group_check=True,
                        )
                        if d == DT - 1:
                            t_ = qt_pool.tile([P, CHUNK], BF16, name=f"qt{j}",
                                              tag=f"qt{j}")
                            nc.vector.tensor_copy(t_[:], ps_q[j][:])
                            qt.append(t_)
                    if c == 0 and d % 4 == 0:
                        k = d // 4
                        qsl = slice(k * QW, (k + 1) * QW)
                        nc.sync.dma_start(out=wv_all[:, qsl], in_=wvT[:, qsl])
                    yield
                qt_of[c] = qt
                ps_v = [ps_main.tile([P, CHUNK], F32, name=f"psv{i}", tag=f"pm{i}")
                        for i in range(4)]
                for d in range(DT):
                    for i in range(4):
                        nc.tensor.matmul(
                            ps_v[i][:],
                            xt[:, d * CHUNK + i * P:d * CHUNK + (i + 1) * P],
                            wv_all[:, d * JL:(d + 1) * JL],
                            start=(d == 0), stop=(d == DT - 1),
                            skip_group_check=True,
                        )
                        if d == DT - 1:
                            nc.vector.tensor_copy(v_t[4 * c + i][:], ps_v[i][:])
                    if c == 0 and d % 8 == 0:
                        half = slice((d // 8) * 2 * D, ((d // 8) + 1) * 2 * D)
                        nc.gpsimd.dma_start(out=wo_all[:, half], in_=woT[:, half])
                    yield
                yield

            def gen_attn_head(c, h):
                # 4G+1 yields, G = c+1
                qt = qt_of[c]
                T = 4 * c + 4
                G = T // 4
                NFULL = 4 * c  # full-width key tiles (before the diagonal)
                rs_acc = ps_rs.tile([1, CHUNK], F32, name="rsacc", tag="rs")
                o_acc = ps_s.tile([P, CHUNK], F32, name="oacc", tag="ss")
                exps = [None] * T
                esum = [None]  # running f32 sum of full-tile exps (vector)

                def emit_scores(t):
                    u = t - 4 * c
                    q0 = u * P if u > 0 else 0
                    ps = ps_s.tile([P, CHUNK], F32, name="pss", tag="ss")
                    nc.tensor.matmul(
                        ps[:, q0:], kT_t[h][:, t * P:(t + 1) * P], qt[h][:, q0:],
                        start=True, stop=True, skip_group_check=True,
                    )
                    e = exp_pool.tile([P, CHUNK], BF16, name="exp", tag="exp")
                    nc.scalar.activation(
                        e[:, q0:], ps[:, q0:], mybir.ActivationFunctionType.Exp,
                        scale=SCALE,
                    )
                    if u >= 0:
                        nc.vector.tensor_mul(
                            e[:, q0:q0 + P], e[:, q0:q0 + P], master[:, 384:384 + P])
                    exps[t] = (e, q0)
                    # fold full-width tiles into the vector-engine exp sum; the
                    # denominator then needs a single ones-matmul instead of
                    # one per key tile.
                    if u < 0 and t >= 1:
                        s_ = esum_pool.tile([P, CHUNK], F32R, name="esum",
                                            tag="esum")
                        if t == 1:
                            nc.vector.tensor_add(s_[:], exps[0][0][:], e[:])
                        else:
                            nc.vector.tensor_add(s_[:], esum[0][:], e[:])
                        esum[0] = s_

                def emit_b(t):
                    e, q0 = exps[t]
                    if t >= NFULL:
                        nc.tensor.matmul(
                            rs_acc[:, q0:], ones[:], e[:, q0:],
                            start=(t == 0), stop=(t == T - 1),
                            skip_group_check=True,
                        )
                    nc.tensor.matmul(
                        o_acc[:, q0:], v_t[t][:, h * P:(h + 1) * P], e[:, q0:],
                        start=(t == 0), stop=(t == T - 1),
                        skip_group_check=True,
                    )

                for g in range(G):
                    emit_scores(4 * g); emit_scores(4 * g + 1)
                    yield
                    emit_scores(4 * g + 2); emit_scores(4 * g + 3)
                    yield
                    if g >= 1:
                        emit_b(4 * (g - 1)); emit_b(4 * (g - 1) + 1)
                        yield
                        emit_b(4 * (g - 1) + 2); emit_b(4 * (g - 1) + 3)
                        yield
                if NFULL > 0:
                    nc.tensor.matmul(
                        rs_acc[:], ones_r[:], esum[0][:],
                        start=True, stop=False, skip_group_check=True,
                    )
                emit_b(4 * (G - 1)); emit_b(4 * (G - 1) + 1)
                yield
                emit_b(4 * (G - 1) + 2); emit_b(4 * (G - 1) + 3)

                # normalize: reciprocal (PSUM->SBUF), broadcast, multiply
                rs_sb = small_pool.tile([1, CHUNK], F32, name="rssb", tag="rssb")
                nc.vector.reciprocal_approx_fast(out=rs_sb[:], in_=rs_acc[:])
                rb = small_pool.tile([P, CHUNK], F32, name="rb", tag="rb")
                nc.gpsimd.partition_broadcast(rb[:], rs_sb[:])
                ot = ot_pool.tile([P, CHUNK], BF16, name="ot", tag="ot")
                nc.vector.tensor_mul(ot[:], o_acc[:], rb[:])
                ots_of.setdefault(c, []).append(ot)
                yield

            def gen_attn(c):
                for h in range(NH):
                    yield from gen_attn_head(c, h)

            def gen_wo(c):
                ots = ots_of.pop(c)
                ob = osb_pool.tile([P, XW], BF16, name="ob", tag="ob")
                for j2 in range(DT):
                    pw = ps_s.tile([P, CHUNK], F32, name="pw", tag="ss")
                    for h in range(NH):
                        nc.tensor.matmul(
                            pw[:],
                            wo_all[:, h * D + j2 * P:h * D + (j2 + 1) * P],
                            ots[h][:],
                            start=(h == 0), stop=(h == NH - 1),
                            skip_group_check=True,
                        )
                    nc.vector.tensor_copy(
                        ob[:, j2 * CHUNK:(j2 + 1) * CHUNK], pw[:])
                    if j2 % 4 == 3:
                        osl = slice((j2 - 3) * CHUNK, (j2 + 1) * CHUNK)
                        nc.gpsimd.dma_start(
                            out=outT[:, c * XW + (j2 - 3) * CHUNK:
                                     c * XW + (j2 + 1) * CHUNK],
                            in_=ob[:, osl])
                    yield

            def drive(g):
                for _ in g:
                    pass

            SENT = object()

            def interleave(gmain, nmain, gsub, nsub):
                im = isub = 0
                main_done = sub_done = False
                while not (main_done and sub_done):
                    go_main = sub_done or (
                        not main_done and im * nsub <= isub * nmain)
                    if go_main:
                        if next(gmain, SENT) is SENT:
                            main_done = True
                        else:
                            im += 1
                    else:
                        if next(gsub, SENT) is SENT:
                            sub_done = True
                        else:
                            isub += 1

            # ---- schedule ----
            drive(gen_kproj(0))
            drive(gen_qv(0))
            for c in range(1, NCH):
                drive(gen_kproj(c))
                interleave(gen_qv(c), 33, gen_attn(c - 1), 16 * c + 4)
                drive(gen_wo(c - 1))
            drive(gen_attn(NCH - 1))
            drive(gen_wo(NCH - 1))

    nc.compile()
    return nc


_NC_CACHE = None


def _get_nc():
    global _NC_CACHE
    if _NC_CACHE is None:
        _NC_CACHE = build_kernel()
    return _NC_CACHE


def _tile128(a):
    """[R, C] -> [128, (R/128)*C] with out[p, r*C + c] = a[r*128 + p, c]."""
    R, C = a.shape
    return np.ascontiguousarray(
        a.reshape(R // P, P, C).transpose(1, 0, 2).reshape(P, -1))


def make_in_maps(x, wq, wk, wv, wo):
    bf16 = ml_dtypes.bfloat16
    in_maps = []
    for core in range(8):
        b, g = core // 4, core % 4
        j0 = g * JL
        # xT tiled [p, c, d, s']: = x[b][c*512+s', d*128+p]
        xb = x[b].astype(bf16)                      # [s, dcol]
        xt = np.ascontiguousarray(
            xb.reshape(NCH, CHUNK, DT, P).transpose(3, 0, 2, 1).reshape(P, -1))
        in_maps.append({
            "xT": xt,
            "wqT": _tile128(wq[j0:j0 + JL, :].T.astype(bf16)),
            "wkT": _tile128(wk[j0:j0 + JL, :].T.astype(bf16)),
            "wvT": _tile128(wv[j0:j0 + JL, :].T.astype(bf16)),
            "woT": _tile128(wo[:, j0:j0 + JL].T.astype(bf16)),
        })
    return in_maps


def kernel(x, freqs_complex=None, mask=None, wq=None, wk=None, wv=None, wo=None,
           **_unused):
    x = np.asarray(x, dtype=np.float32)
    wq = np.asarray(wq, dtype=np.float32)
    wk = np.asarray(wk, dtype=np.float32)
    wv = np.asarray(wv, dtype=np.float32)
    wo = np.asarray(wo, dtype=np.float32)

    nc = _get_nc()
    in_maps = make_in_maps(x, wq, wk, wv, wo)
    res = run_bass_kernel_spmd(nc, in_maps, list(range(8)))

    out = np.zeros((B, S, D), dtype=np.float32)
    for core in range(8):
        # outT tiled [p, c, j2, s'] -> out[b][c*512+s', j2*128+p]
        arr = np.asarray(res.results[core]["outT"]).astype(np.float32)
        out[core // 4] += arr.reshape(P, NCH, DT, CHUNK).transpose(
            1, 3, 2, 0).reshape(S, D)
    return out
